# revision 1
# baseline (speedup 1.0000x reference)
"""Multi-head differential attention on 8 Trainium2 NeuronCores.

Sharding: core c -> batch c//4, head-group c%4 (4 of 16 heads).
Per core: QKV projection for its heads, k-major attention (scores
transposed; softmax denominators come from a ones-row appended to V via
the AV matmul), per-(batch,head) softmax normalization and GroupNorm
statistics.  The *pre-affine* normalized tensor z is AllGathered in bf16
(pair 0's gather hides under pair 1's attention); the GroupNorm affine
(mean/rstd per head) rides the second gather as bitcast payload columns
and is folded on-device into a scaled Wo and a constant bias row.  Each
core then runs a column-parallel out-projection producing a 256-column
slice of the output, assembled on host.

Host side folds: lambda and softmax scale into Wq/bq; GroupNorm affine
into Wo/bo.  x is pre-transposed per batch and cast to bf16 so all
matmuls run at 1 cycle/row.
"""

import numpy as np
import ml_dtypes

B, S, D, H, DH = 2, 2048, 1024, 16, 64
HPC = 4            # heads per core
CW = HPC * DH      # attention columns per core (256)
EPS = 1e-5
LAMBDA_INIT = 0.8
N_CORES = 8
SCC = 16           # scalar payload columns (8 f32 as 16 bf16)

_cache = {}


def _build(with_collective=True):
    from contextlib import ExitStack
    import concourse.bass as bass
    from concourse import bacc
    import concourse.tile as tile
    import concourse.mybir as mybir

    f32 = mybir.dt.float32
    bf16 = mybir.dt.bfloat16
    AF = mybir.ActivationFunctionType
    ALU = mybir.AluOpType

    nc = bacc.Bacc("TRN2", target_bir_lowering=False, debug=False,
                   num_devices=N_CORES)

    xt_d = nc.dram_tensor("xt", [D, S], bf16, kind="ExternalInput")
    wq_d = nc.dram_tensor("wq", [D, CW], bf16, kind="ExternalInput")
    wk_d = nc.dram_tensor("wk", [D, CW], bf16, kind="ExternalInput")
    wv_d = nc.dram_tensor("wv", [D, CW], bf16, kind="ExternalInput")
    # wo: gathered-row layout [(g t o p), quarter-cols]
    wo_d = nc.dram_tensor("wo", [D, CW], bf16, kind="ExternalInput")
    bq_d = nc.dram_tensor("bq", [CW], bf16, kind="ExternalInput")
    bk_d = nc.dram_tensor("bk", [CW], bf16, kind="ExternalInput")
    bv_d = nc.dram_tensor("bv", [CW], f32, kind="ExternalInput")
    bvf_d = nc.dram_tensor("bvf", [D], f32, kind="ExternalInput")
    bo_d = nc.dram_tensor("bo", [CW], bf16, kind="ExternalInput")
    y_d = nc.dram_tensor("y", [2, 128, S], f32, kind="ExternalOutput")

    ag_in0 = nc.dram_tensor("ag_in0", [128, S], bf16)
    ag_in1 = nc.dram_tensor("ag_in1", [128, S + SCC], bf16)
    ag_out0 = nc.dram_tensor("ag_out0", [4, 128, S], bf16)
    ag_out1 = nc.dram_tensor("ag_out1", [4, 128, S + SCC], bf16)
    rs_d = nc.dram_tensor("rs_scratch", [HPC, S], f32)

    NQT = 4          # query tiles of 512
    QT = 512
    NKT = 16         # key tiles of 128
    NDC = 8          # d-chunks of 128

    with ExitStack() as ctx:
        tc = ctx.enter_context(tile.TileContext(nc))
        const = ctx.enter_context(tc.tile_pool(name="const", bufs=1))
        big = ctx.enter_context(tc.tile_pool(name="big", bufs=1))

        pxt = ctx.enter_context(tc.tile_pool(name="pxt", bufs=1))
        xt_sb = [pxt.tile([128, S], bf16, tag=f"xt{c}", name=f"xt{c}")
                 for c in range(NDC)]
        for c in range(NDC):
            nc.sync.dma_start(out=xt_sb[c], in_=xt_d[c * 128:(c + 1) * 128, :])

        # ---- constants ----
        wq_sb = const.tile([128, NDC, CW], bf16, tag="wq")
        wk_sb = const.tile([128, NDC, CW], bf16, tag="wk")
        wv_sb = const.tile([128, NDC, CW], bf16, tag="wv")
        nc.sync.dma_start(out=wq_sb, in_=wq_d[:, :].rearrange("(c p) n -> p c n", p=128))
        nc.sync.dma_start(out=wk_sb, in_=wk_d[:, :].rearrange("(c p) n -> p c n", p=128))
        nc.sync.dma_start(out=wv_sb, in_=wv_d[:, :].rearrange("(c p) n -> p c n", p=128))
        wo_sb = const.tile([128, NDC, CW], bf16, tag="wo")
        nc.sync.dma_start(out=wo_sb, in_=wo_d[:, :].rearrange("(c p) n -> p c n", p=128))

        bqr_sb = const.tile([1, CW], bf16, tag="bqr")
        bkr_sb = const.tile([1, CW], bf16, tag="bkr")
        bor_sb = const.tile([1, CW], bf16, tag="bor")
        nc.sync.dma_start(out=bqr_sb, in_=bq_d[:].rearrange("(a n) -> a n", a=1))
        nc.sync.dma_start(out=bkr_sb, in_=bk_d[:].rearrange("(a n) -> a n", a=1))
        nc.sync.dma_start(out=bor_sb, in_=bo_d[:].rearrange("(a n) -> a n", a=1))
        bv0_sb = const.tile([64, HPC], f32, tag="bv0")
        nc.sync.dma_start(out=bv0_sb, in_=bv_d[:].rearrange("(h p) -> p h", p=64))
        bv_sb = const.tile([64, HPC], f32, tag="bv")
        nc.vector.tensor_copy(bv_sb, bv0_sb)  # pre-touch: keep deps DVE-local

        onesrow_sb = const.tile([1, QT], bf16, tag="onesrow")
        nc.vector.memset(onesrow_sb, 1.0)
        ones_sb = const.tile([64, 1], f32, tag="ones")
        nc.vector.memset(ones_sb, 1.0)
        ones2_sb = const.tile([2, 64], f32, tag="ones2")
        nc.vector.memset(ones2_sb, 1.0)

        qT_sb = big.tile([128, 2, S], bf16, tag="qT")   # pair t: head 2t rows 0:64
        kT_sb = big.tile([128, 2, S], bf16, tag="kT")
        v_sb = [big.tile([128, NKT, DH + 1], bf16, tag=f"v{h}", name=f"v{h}")
                for h in range(HPC)]
        z_sb = [big.tile([DH + 1, S], f32, tag=f"z{h}", name=f"z{h}")
                for h in range(HPC)]
        zp_sb = [big.tile([128, S + SCC], bf16, tag=f"zp{t}", name=f"zp{t}")
                 for t in range(2)]

        # ---- Phase B: QKV projections (pair 0 first so attention starts early)
        with tc.tile_pool(name="pbqk", bufs=4, space="PSUM") as pbqk, \
             tc.tile_pool(name="pbv", bufs=3, space="PSUM") as pbv:

            def qk_proj(t, w_sb, br_sb, dst):
                pss = [pbqk.tile([128, QT], f32, tag="qk",
                                 name=f"qk{t}{st}{w_sb.tensor.name}")
                       for st in range(NQT)]
                for c in range(NDC):
                    for st in range(NQT):
                        nc.tensor.matmul(pss[st], w_sb[:, c, t * 128:(t + 1) * 128],
                                         xt_sb[c][:, st * QT:(st + 1) * QT],
                                         start=(c == 0), stop=False)
                for st in range(NQT):
                    nc.tensor.matmul(pss[st], br_sb[:, t * 128:(t + 1) * 128],
                                     onesrow_sb, start=False, stop=True)
                    nc.vector.tensor_copy(out=dst[:, t, st * QT:(st + 1) * QT],
                                          in_=pss[st])

            qk_proj(0, wq_sb, bqr_sb, qT_sb)
            qk_proj(0, wk_sb, bkr_sb, kT_sb)
            for st in range(NKT):
                ps = pbv.tile([128, CW], f32, tag="v", name=f"vv{st}")
                for c in range(NDC):
                    nc.tensor.matmul(ps, xt_sb[c][:, st * 128:(st + 1) * 128],
                                     wv_sb[:, c, :],
                                     start=(c == 0), stop=(c == NDC - 1))
                for h in range(HPC):
                    nc.vector.tensor_copy(out=v_sb[h][:, st, 0:DH],
                                          in_=ps[:, h * DH:(h + 1) * DH])
            for h in range(HPC):
                nc.vector.memset(v_sb[h][:, :, DH:DH + 1], 1.0)
            qk_proj(1, wq_sb, bqr_sb, qT_sb)
            qk_proj(1, wk_sb, bkr_sb, kT_sb)

        # ---- Phase C: attention per head-pair; softmax-normalize, stats,
        #      and the pair's AllGather all overlap the next pair ----
        with tc.tile_pool(name="psc", bufs=2, space="PSUM") as psc, \
             tc.tile_pool(name="pav", bufs=4, space="PSUM") as pav, \
             tc.tile_pool(name="pexp", bufs=4) as pexp, \
             tc.tile_pool(name="pd", bufs=1) as pd:
            bnst = [pd.tile([64, NQT, 6], f32, tag=f"bn{h}", name=f"bnst{h}")
                    for h in range(HPC)]
            vr_all = pd.tile([1, HPC], f32, tag="vr_all", name="vr_all")
            msc_all = pd.tile([1, 2 * HPC], f32, tag="msc", name="msc_all")
            stk_all = [pd.tile([64, 3], f32, tag=f"stk{h}", name=f"stk{h}")
                       for h in range(HPC)]
            for t in range(2):
                h0, h1 = 2 * t, 2 * t + 1
                for qt in range(NQT):
                    av0 = pav.tile([DH + 1, QT], f32, tag="av", name=f"av{t}{qt}a")
                    av1 = pav.tile([DH + 1, QT], f32, tag="av", name=f"av{t}{qt}b")
                    for kt in range(NKT):
                        sps = psc.tile([128, 2 * QT], f32, tag="s", name=f"s{t}{qt}{kt}")
                        for o in range(2):
                            nc.tensor.matmul(
                                sps[:, o * QT:(o + 1) * QT],
                                kT_sb[64 * o:64 * (o + 1), t, kt * 128:(kt + 1) * 128],
                                qT_sb[64 * o:64 * (o + 1), t, qt * QT:(qt + 1) * QT],
                                start=True, stop=True)
                        e_sb = pexp.tile([128, 2 * QT], bf16, tag="e", name=f"e{t}{qt}{kt}")
                        nc.scalar.activation(e_sb, sps, AF.Exp)
                        nc.tensor.matmul(av0, v_sb[h0][:, kt, :], e_sb[:, 0:QT],
                                         start=(kt == 0), stop=(kt == NKT - 1))
                        nc.tensor.matmul(av1, v_sb[h1][:, kt, :], e_sb[:, QT:2 * QT],
                                         start=(kt == 0), stop=(kt == NKT - 1))
                    nc.vector.tensor_copy(out=z_sb[h0][:, qt * QT:(qt + 1) * QT], in_=av0)
                    nc.vector.tensor_copy(out=z_sb[h1][:, qt * QT:(qt + 1) * QT], in_=av1)

                # softmax normalize + GN stats for this pair (DVE/DMA only:
                # no PE instructions, so pair 1's matmuls are never blocked)
                for i, h in enumerate((h0, h1)):
                    nc.sync.dma_start(out=rs_d[h:h + 1, :], in_=z_sb[h][DH:DH + 1, :])
                    rb = pd.tile([64, S], f32, tag="rb", bufs=2, name=f"rb{h}")
                    nc.gpsimd.dma_start(out=rb,
                                        in_=rs_d[h:h + 1, :].to_broadcast([64, S]))
                    nc.vector.reciprocal_approx_fast(rb, rb)
                    nc.vector.tensor_mul(z_sb[h][0:DH, :], z_sb[h][0:DH, :], rb)
                    # assemble the gather payload (bf16): even head rows 0:64
                    # via DVE cast, odd head rows 64:128 via DMA (cross-part)
                    if i == 0:
                        nc.vector.tensor_copy(out=zp_sb[t][0:64, 0:S],
                                              in_=z_sb[h][0:DH, :])
                    else:
                        nc.gpsimd.dma_start(out=zp_sb[t][64:128, 0:S],
                                            in_=z_sb[h][0:DH, :])
                    for st in range(NQT):
                        nc.vector.bn_stats(out=bnst[h][:, st, :],
                                           in_=z_sb[h][0:DH, st * QT:(st + 1) * QT])
                    mvh = pd.tile([64, 2], f32, tag="mv", bufs=2, name=f"mv{h}")
                    nc.vector.bn_aggr(out=mvh, in_=bnst[h])
                    stk = stk_all[h]
                    nc.vector.tensor_add(stk[:, 0:1], mvh[:, 0:1], bv_sb[:, h:h + 1])
                    nc.vector.tensor_copy(stk[:, 1:2], mvh[:, 1:2])
                    nc.vector.tensor_mul(stk[:, 2:3], stk[:, 0:1], stk[:, 0:1])

                if t == 0:
                    # pair 0: gather z now -- fully hidden under pair 1
                    nc.sync.dma_start(out=ag_in0[:, :], in_=zp_sb[0][:, 0:S])
                    if with_collective:
                        nc.gpsimd.collective_compute(
                            "AllGather", ALU.bypass,
                            replica_groups=[[0, 1, 2, 3], [4, 5, 6, 7]],
                            ins=[ag_in0[:].opt()],
                            outs=[ag_out0[:].opt()],
                        )
                    else:
                        for g in range(4):
                            nc.sync.dma_start(out=ag_out0[g], in_=ag_in0[:, :])

            # ---- GN scalar tail: one sqrt table switch total ----
            scg = pd.tile([1, HPC, 3], f32, tag="scg", name="scg")
            for h in range(HPC):
                stp = pav.tile([1, 3], f32, tag="av", name=f"stp{h}")
                nc.tensor.matmul(stp, ones_sb, stk_all[h], start=True, stop=True)
                nc.vector.tensor_copy(scg[:, h, :], stp)
            e3 = pd.tile([1, HPC, 3], f32, tag="e3", name="e3")
            nc.vector.tensor_scalar(out=e3, in0=scg, scalar1=1.0 / 64.0,
                                    scalar2=None, op0=ALU.mult)
            m2 = pd.tile([1, HPC], f32, tag="m2", name="m2")
            nc.vector.tensor_mul(m2, e3[:, :, 0], e3[:, :, 0])
            nc.vector.tensor_add(vr_all, e3[:, :, 1], e3[:, :, 2])
            nc.vector.tensor_tensor(out=vr_all, in0=vr_all, in1=m2, op=ALU.subtract)
            eps_t = pd.tile([1, 1], f32, tag="eps", name="eps_t")
            nc.vector.memset(eps_t, EPS)
            sd_all = pd.tile([1, HPC], f32, tag="sd", name="sd_all")
            nc.scalar.activation(sd_all, vr_all, AF.Sqrt, bias=eps_t)
            rr = pd.tile([1, HPC], f32, tag="rr", name="rr")
            nc.vector.reciprocal(rr, sd_all)
            # parity-major payload order: [M0,M2,M1,M3, r0,r2,r1,r3]
            for j, h in enumerate((0, 2, 1, 3)):
                nc.vector.tensor_copy(msc_all[:, j:j + 1], e3[:, h, 0:1])
                nc.vector.tensor_copy(msc_all[:, HPC + j:HPC + j + 1],
                                      rr[:, h:h + 1])

            # scalars ride the pair-1 gather as bitcast bf16 payload columns
            nc.vector.tensor_copy(out=zp_sb[1][0:1, S:S + SCC],
                                  in_=msc_all[0:1, :].bitcast(bf16))
            nc.sync.dma_start(out=ag_in1[:, :], in_=zp_sb[1][:, :])
            if with_collective:
                nc.gpsimd.collective_compute(
                    "AllGather", ALU.bypass,
                    replica_groups=[[0, 1, 2, 3], [4, 5, 6, 7]],
                    ins=[ag_in1[:].opt()],
                    outs=[ag_out1[:].opt()],
                )
            else:
                for g in range(4):
                    nc.sync.dma_start(out=ag_out1[g], in_=ag_in1[:, :])

        # ---- Phase E: fold GN affine into Wo, column-parallel out-proj ----
        with tc.tile_pool(name="pg", bufs=1) as pg, \
             tc.tile_pool(name="pf", bufs=4, space="PSUM") as pf, \
             tc.tile_pool(name="pystage", bufs=2) as pystage:
            # gathered scalars: [4 groups, 8 f32] as bitcast bf16 rows
            sc16 = pg.tile([1, 4, SCC], bf16, tag="sc16")
            nc.sync.dma_start(
                out=sc16,
                in_=ag_out1[:, 0:1, S:S + SCC].rearrange("g p c -> p g c"))
            # [1, 4, 8] f32 per group: (M0,M2,M1,M3, r0,r2,r1,r3)
            scf = sc16[:, :, :].bitcast(f32)
            # per out-partition half o: values for chunks c=(g,t) are the
            # parity-o pair within each group -- contiguous slices
            rstg = pg.tile([1, 2, 4, 2], f32, tag="rstg")
            mstg = pg.tile([1, 2, 4, 2], f32, tag="mstg")
            for o in range(2):
                nc.vector.tensor_copy(out=rstg[:, o, :, :],
                                      in_=scf[:, :, HPC + 2 * o:HPC + 2 * o + 2])
                nc.vector.tensor_copy(out=mstg[:, o, :, :],
                                      in_=scf[:, :, 2 * o:2 * o + 2])
            s2p = pf.tile([128, NDC], f32, tag="s2p", bufs=1, name="s2p")
            mcp = pf.tile([128, NDC], f32, tag="mcp", bufs=1, name="mcp")
            for o in range(2):
                nc.tensor.matmul(s2p[64 * o:64 * (o + 1), :], ones2_sb[0:1, :],
                                 rstg[:, o, :, :], start=True, stop=True)
                nc.tensor.matmul(mcp[64 * o:64 * (o + 1), :], ones2_sb[0:1, :],
                                 mstg[:, o, :, :], start=True, stop=True)
            s2c = pg.tile([128, NDC], f32, tag="s2c")
            nc.vector.tensor_copy(s2c, s2p)
            bvg = pg.tile([128, NDC], f32, tag="bvg")
            nc.sync.dma_start(out=bvg, in_=bvf_d[:].rearrange("(c p) -> p c", p=128))
            mcs = pg.tile([128, NDC], f32, tag="mcs")
            nc.vector.tensor_tensor(out=mcs, in0=mcp, in1=bvg, op=ALU.subtract)
            mvec = pg.tile([128, NDC], bf16, tag="mvec")
            nc.vector.tensor_mul(mvec, mcs, s2c)

            # wo_scaled[p, (c,n)] = r_head(p,c) * wo ; cst[n] = sum_p M*r*wo
            wos = pg.tile([128, NDC, CW], bf16, tag="wos")
            for c in range(NDC):
                nc.vector.tensor_scalar(out=wos[:, c, :], in0=wo_sb[:, c, :],
                                        scalar1=s2c[:, c:c + 1], scalar2=None,
                                        op0=ALU.mult)
            cstp = pf.tile([1, CW], f32, tag="cst", bufs=1, name="cstp")
            for c in range(NDC):
                nc.tensor.matmul(cstp, mvec[:, c:c + 1], wo_sb[:, c, :],
                                 start=(c == 0), stop=(c == NDC - 1))
            brow = pg.tile([1, CW], bf16, tag="brow")
            nc.vector.tensor_tensor(out=brow, in0=bor_sb, in1=cstp, op=ALU.subtract)

            nrmg_sb = pg.tile([128, NDC, S], bf16, tag="nrmg")
            for g in range(4):
                nc.sync.dma_start(out=nrmg_sb[:, 2 * g, :], in_=ag_out0[g, :, :])
                nc.sync.dma_start(out=nrmg_sb[:, 2 * g + 1, :],
                                  in_=ag_out1[g, :, 0:S])

            for nt in range(2):
                ystage = pystage.tile([128, S], f32, tag="ys", name=f"ys{nt}")
                yps = [pf.tile([128, QT], f32, tag="y", name=f"yp{nt}{st}")
                       for st in range(NQT)]
                for c in range(NDC):
                    for st in range(NQT):
                        nc.tensor.matmul(yps[st], wos[:, c, nt * 128:(nt + 1) * 128],
                                         nrmg_sb[:, c, st * QT:(st + 1) * QT],
                                         start=(c == 0), stop=False)
                for st in range(NQT):
                    nc.tensor.matmul(yps[st], brow[:, nt * 128:(nt + 1) * 128],
                                     onesrow_sb, start=False, stop=True)
                    nc.scalar.activation(ystage[:, st * QT:(st + 1) * QT], yps[st],
                                         AF.Copy)
                nc.sync.dma_start(out=y_d[nt, :, :], in_=ystage)

    nc.compile()
    return nc


def _get_nc():
    if "nc" not in _cache:
        _cache["nc"] = _build()
    return _cache["nc"]


def _host_prep(x, Wq, bq, Wk, bk, Wv, bv, Wo, bo, lq1, lk1, lq2, lk2, gn_w, gn_b):
    x = np.asarray(x, np.float32)
    lam = (np.exp((np.asarray(lq1) * np.asarray(lk1)).sum(-1))
           - np.exp((np.asarray(lq2) * np.asarray(lk2)).sum(-1)) + LAMBDA_INIT)
    qscale = (DH ** -0.5) * lam
    Wq_eff = (np.asarray(Wq).reshape(D, H, DH) * qscale[None, :, None]).reshape(D, D)
    bq_eff = (np.asarray(bq).reshape(H, DH) * qscale[:, None]).reshape(D)
    gw = np.asarray(gn_w).reshape(D)
    gb = np.asarray(gn_b).reshape(D)
    Wo_eff = np.asarray(Wo) * gw[:, None]
    bo_eff = np.asarray(bo) + gb @ np.asarray(Wo)

    # Gathered-row order (chunk (g,t), partition (o,dh) -> head 4g+2t+o) is
    # exactly the original row-major head order, so Wo_eff rows need no
    # permutation.
    xT = np.ascontiguousarray(x.transpose(0, 2, 1))  # [B, D, S]
    bf = ml_dtypes.bfloat16

    in_maps = []
    for c in range(N_CORES):
        b, hg = c // 4, c % 4
        cs = slice(CW * hg, CW * (hg + 1))
        in_maps.append({
            "xt": np.ascontiguousarray(xT[b]).astype(bf),
            "wq": np.ascontiguousarray(Wq_eff[:, cs]).astype(bf),
            "wk": np.ascontiguousarray(np.asarray(Wk)[:, cs]).astype(bf),
            "wv": np.ascontiguousarray(np.asarray(Wv)[:, cs]).astype(bf),
            "wo": np.ascontiguousarray(Wo_eff[:, cs]).astype(bf),
            "bq": np.ascontiguousarray(bq_eff[cs]).astype(bf),
            "bk": np.ascontiguousarray(np.asarray(bk)[cs]).astype(bf),
            "bv": np.ascontiguousarray(np.asarray(bv)[cs]).astype(np.float32),
            "bvf": np.ascontiguousarray(np.asarray(bv)).astype(np.float32),
            "bo": np.ascontiguousarray(bo_eff[cs]).astype(bf),
        })
    return in_maps


def _host_gather(outs):
    # core c=4b+hg produced output columns [256*hg, 256*(hg+1)) as [2,128,S]
    yT = np.empty((B, D, S), np.float32)
    for b in range(B):
        for hg in range(4):
            q = np.asarray(outs[4 * b + hg]["y"]).reshape(CW, S)
            yT[b, CW * hg:CW * (hg + 1), :] = q
    return np.ascontiguousarray(yT.transpose(0, 2, 1))


def kernel(x, Wq, bq, Wk, bk, Wv, bv, Wo, bo, lq1, lk1, lq2, lk2, gn_w, gn_b):
    from concourse.bass_utils import run_bass_kernel_spmd

    in_maps = _host_prep(x, Wq, bq, Wk, bk, Wv, bv, Wo, bo,
                         lq1, lk1, lq2, lk2, gn_w, gn_b)
    nc = _get_nc()
    res = run_bass_kernel_spmd(nc, in_maps, core_ids=list(range(N_CORES)))
    return _host_gather(res.results)



# revision 6
# speedup vs baseline: 1.0397x; 1.0397x over previous
"""Multi-head differential attention on 8 Trainium2 NeuronCores.

Sharding: core c -> batch c//4, head-group c%4 (4 of 16 heads).

v2 pipeline (vs baseline): the softmax exp stream on the Scalar engine is
the per-core floor (~128us), so everything is scheduled around keeping it
fed from t~14us onward:
  - prefix computes only q/k of head-pair 0; V and pair-1 q/k are emitted
    as PE filler inside pair-0's attention loop (the PE has slack while
    ACT chews exps).
  - attention inner loop is kt-pipelined: score pair (row-tiled 64x128
    auto-tiles) -> exp of [128,1024] PSUM tile -> e_sb (bf16, 16 kt tiles
    per (t,qt)) -> deferred AV (lag 2) accumulating z+denominator via the
    DH+1 ones-row.
  - softmax-normalize + GroupNorm stats + gather payload happen per
    (t,qt), and the z AllGather is split into 8 per-(t,qt) chunks that
    pipeline on the CC rings under the attention phase (the monolithic
    per-pair gathers were 2x57us, mostly exposed).
  - k-bias is dropped entirely (constant along the softmax axis), lambda
    and softmax scale are folded into Wq/bq, GroupNorm affine into Wo/bo
    on host; rstd uses exp(-0.5*ln(var+eps)) so the whole kernel needs
    one ACT table set (no mid-kernel sqrt table switch).
Each core then runs a column-parallel out-projection producing a
256-column slice of the output, assembled on host.
"""

import numpy as np
import ml_dtypes

B, S, D, H, DH = 2, 2048, 1024, 16, 64
HPC = 4            # heads per core
CW = HPC * DH      # attention columns per core (256)
EPS = 1e-5
LAMBDA_INIT = 0.8
N_CORES = 8
SCC = 16           # scalar payload columns (8 f32 as 16 bf16)
QT = 512           # q-block per (t, qt)
NQT = 4
NKT = 16
NDC = 8

_cache = {}


def _build(with_collective=True):
    from contextlib import ExitStack
    import concourse.bass as bass
    from concourse import bacc
    import concourse.tile as tile
    import concourse.mybir as mybir

    f32 = mybir.dt.float32
    bf16 = mybir.dt.bfloat16
    AF = mybir.ActivationFunctionType
    ALU = mybir.AluOpType

    nc = bacc.Bacc("TRN2", target_bir_lowering=False, debug=False,
                   num_devices=N_CORES)

    xt_d = nc.dram_tensor("xt", [D, S], bf16, kind="ExternalInput")
    wq_d = nc.dram_tensor("wq", [D, CW], bf16, kind="ExternalInput")
    wk_d = nc.dram_tensor("wk", [D, CW], bf16, kind="ExternalInput")
    wv_d = nc.dram_tensor("wv", [D, CW], bf16, kind="ExternalInput")
    wo_d = nc.dram_tensor("wo", [D, CW], bf16, kind="ExternalInput")
    bq_d = nc.dram_tensor("bq", [CW], bf16, kind="ExternalInput")
    bv_d = nc.dram_tensor("bv", [CW], f32, kind="ExternalInput")
    bvf_d = nc.dram_tensor("bvf", [D], f32, kind="ExternalInput")
    bo_d = nc.dram_tensor("bo", [CW], bf16, kind="ExternalInput")
    y_d = nc.dram_tensor("y", [2, 128, S], f32, kind="ExternalOutput")

    rs_d = nc.dram_tensor("rs_scratch", [2, 2, NQT, QT], f32)
    warm_in = nc.dram_tensor("warm_in", [1, 64], bf16)
    warm_out = nc.dram_tensor("warm_out", [4, 1, 64], bf16)
    ag_in = nc.dram_tensor("ag_in", [2, NQT, 128, QT], bf16)
    ag_in_last = nc.dram_tensor("ag_in_last", [128, QT + SCC], bf16)
    ag_out = nc.dram_tensor("ag_out", [2, NQT, 4, 128, QT], bf16)
    ag_out_last = nc.dram_tensor("ag_out_last", [4, 128, QT + SCC], bf16)

    GROUPS = [[0, 1, 2, 3], [4, 5, 6, 7]]

    def gather(in_ap, out_ap):
        if with_collective:
            nc.gpsimd.collective_compute(
                "AllGather", ALU.bypass, replica_groups=GROUPS,
                ins=[in_ap.opt()], outs=[out_ap.opt()])
        else:
            for g in range(4):
                nc.sync.dma_start(out=out_ap[g], in_=in_ap)

    with ExitStack() as ctx:
        tc = ctx.enter_context(tile.TileContext(nc))
        const = ctx.enter_context(tc.tile_pool(name="const", bufs=1))
        big = ctx.enter_context(tc.tile_pool(name="big", bufs=1))

        pxt = ctx.enter_context(tc.tile_pool(name="pxt", bufs=1))
        xt_sb = [pxt.tile([128, S], bf16, tag=f"xt{c}", name=f"xt{c}")
                 for c in range(NDC)]
        for c in range(NDC):
            nc.sync.dma_start(out=xt_sb[c], in_=xt_d[c * 128:(c + 1) * 128, :])

        # ---- constants ----
        wq_sb = const.tile([128, NDC, CW], bf16, tag="wq")
        wk_sb = const.tile([128, NDC, CW], bf16, tag="wk")
        wv_sb = const.tile([128, NDC, CW], bf16, tag="wv")
        wo_sb = const.tile([128, NDC, CW], bf16, tag="wo")
        nc.sync.dma_start(out=wq_sb, in_=wq_d[:, :].rearrange("(c p) n -> p c n", p=128))
        nc.sync.dma_start(out=wk_sb, in_=wk_d[:, :].rearrange("(c p) n -> p c n", p=128))
        nc.sync.dma_start(out=wv_sb, in_=wv_d[:, :].rearrange("(c p) n -> p c n", p=128))
        nc.sync.dma_start(out=wo_sb, in_=wo_d[:, :].rearrange("(c p) n -> p c n", p=128))

        bqr_sb = const.tile([1, CW], bf16, tag="bqr")
        bor_sb = const.tile([1, CW], bf16, tag="bor")
        nc.sync.dma_start(out=bqr_sb, in_=bq_d[:].rearrange("(a n) -> a n", a=1))
        nc.sync.dma_start(out=bor_sb, in_=bo_d[:].rearrange("(a n) -> a n", a=1))
        bv0_sb = const.tile([64, HPC], f32, tag="bv0")
        nc.sync.dma_start(out=bv0_sb, in_=bv_d[:].rearrange("(h p) -> p h", p=64))
        bv_sb = const.tile([64, HPC], f32, tag="bv")
        nc.vector.tensor_copy(bv_sb, bv0_sb)  # pre-touch: keep deps DVE-local

        onesrow_sb = const.tile([1, QT], bf16, tag="onesrow")
        nc.vector.memset(onesrow_sb, 1.0)
        ones_sb = const.tile([64, 1], f32, tag="ones")
        nc.vector.memset(ones_sb, 1.0)
        ones2_sb = const.tile([2, 64], f32, tag="ones2")
        nc.vector.memset(ones2_sb, 1.0)
        wu_sb = const.tile([1, 64], bf16, tag="wu")
        nc.vector.memset(wu_sb, 0.0)

        # early dummy collective: absorbs the cross-core rendezvous barrier
        # under the prefix instead of delaying the first real gather
        nc.sync.dma_start(out=warm_in[:, :], in_=wu_sb)
        gather(warm_in[:], warm_out[:])

        qT_sb = big.tile([128, 2, S], bf16, tag="qT")   # pair t: head 2t rows 0:64
        kT_sb = big.tile([128, 2, S], bf16, tag="kT")
        v_sb = [big.tile([128, NKT, DH + 1], bf16, tag=f"v{h}", name=f"v{h}")
                for h in range(HPC)]
        for h in range(HPC):
            nc.vector.memset(v_sb[h][:, :, DH:DH + 1], 1.0)
        nrmg_sb = big.tile([128, NDC, S], bf16, tag="nrmg")

        with tc.tile_pool(name="pqk", bufs=2, space="PSUM") as pqk, \
             tc.tile_pool(name="psc", bufs=2, space="PSUM") as psc, \
             tc.tile_pool(name="pav", bufs=1, space="PSUM") as pav, \
             tc.tile_pool(name="pe", bufs=1) as pe_pool, \
             tc.tile_pool(name="pnrm", bufs=2) as pnrm, \
             tc.tile_pool(name="pd", bufs=1) as pd:

            bnst = [pd.tile([64, NQT, 6], f32, tag=f"bn{h}", name=f"bnst{h}")
                    for h in range(HPC)]
            vr_all = pd.tile([1, HPC], f32, tag="vr_all", name="vr_all")
            msc_all = pd.tile([1, 2 * HPC], f32, tag="msc", name="msc_all")
            stk_all = [pd.tile([64, 3], f32, tag=f"stk{h}", name=f"stk{h}")
                       for h in range(HPC)]

            def qk_proj_st(t, st, w_sb, bias):
                ps = pqk.tile([128, QT], f32, tag="qk", name=f"qk{t}{st}{w_sb.tensor.name}")
                for c in range(NDC):
                    nc.tensor.matmul(ps, w_sb[:, c, t * 128:(t + 1) * 128],
                                     xt_sb[c][:, st * QT:(st + 1) * QT],
                                     start=(c == 0),
                                     stop=(not bias and c == NDC - 1))
                if bias:
                    nc.tensor.matmul(ps, bqr_sb[:, t * 128:(t + 1) * 128],
                                     onesrow_sb, start=False, stop=True)
                dst = qT_sb if bias else kT_sb
                nc.vector.tensor_copy(out=dst[:, t, st * QT:(st + 1) * QT], in_=ps)

            def v_unit(pair, st):
                # V for heads (2*pair, 2*pair+1) at key block st
                ps = pqk.tile([128, QT], f32, tag="qk", name=f"v{pair}{st}")
                for c in range(NDC):
                    nc.tensor.matmul(ps[:, 0:128],
                                     xt_sb[c][:, st * 128:(st + 1) * 128],
                                     wv_sb[:, c, pair * 128:(pair + 1) * 128],
                                     start=(c == 0), stop=(c == NDC - 1))
                for i in range(2):
                    h = 2 * pair + i
                    nc.vector.tensor_copy(out=v_sb[h][:, st, 0:DH],
                                          in_=ps[:, i * DH:(i + 1) * DH])

            # ---- prefix: q/k for pair 0 only ----
            for st in range(NQT):
                qk_proj_st(0, st, wq_sb, True)
            for st in range(NQT):
                qk_proj_st(0, st, wk_sb, False)

            # deferred PE work, emitted as filler inside pair-0's loop
            fillers = []
            for st in range(NKT):
                fillers.append((1100, lambda st=st: v_unit(0, st)))
            for st in range(NQT):
                fillers.append((4200, lambda st=st: qk_proj_st(1, st, wk_sb, False)))
            for st in range(NQT):
                fillers.append((4700, lambda st=st: qk_proj_st(1, st, wq_sb, True)))
            for st in range(NKT):
                fillers.append((1100, lambda st=st: v_unit(1, st)))
            filler_total = sum(c for c, _ in fillers)
            filler_spent = 0
            filler_budget = 0.0

            for t in range(2):
                h0, h1 = 2 * t, 2 * t + 1
                for qt in range(NQT):
                    e_sb = pe_pool.tile([128, NKT * 2 * QT], bf16,
                                        tag=f"e{qt % 2}", name=f"e{t}{qt}")
                    av = [pav.tile([DH + 1, QT], f32, tag=f"av{i}",
                                   name=f"av{t}{qt}{i}") for i in range(2)]

                    def emit_av(kt):
                        for i in range(2):
                            nc.tensor.matmul(
                                av[i], v_sb[2 * t + i][:, kt, :],
                                e_sb[:, kt * 1024 + i * QT:kt * 1024 + (i + 1) * QT],
                                start=(kt == 0), stop=(kt == NKT - 1))

                    for kt in range(NKT):
                        sps = psc.tile([128, 2 * QT], f32, tag="s",
                                       name=f"s{t}{qt}{kt}")
                        for o in range(2):
                            nc.tensor.matmul(
                                sps[:, o * QT:(o + 1) * QT],
                                kT_sb[64 * o:64 * (o + 1), t, kt * 128:(kt + 1) * 128],
                                qT_sb[64 * o:64 * (o + 1), t, qt * QT:(qt + 1) * QT],
                                start=True, stop=True)
                        nc.scalar.activation(
                            e_sb[:, kt * 1024:(kt + 1) * 1024], sps, AF.Exp)
                        if t == 0:
                            filler_budget += filler_total / 64.0
                            while fillers and filler_spent < filler_budget:
                                cost, fn = fillers.pop(0)
                                fn()
                                filler_spent += cost
                        if kt >= 2:
                            emit_av(kt - 2)
                    emit_av(NKT - 2)
                    emit_av(NKT - 1)

                    # ---- per-(t,qt) softmax normalize + GN stats + gather ----
                    last = (t == 1 and qt == NQT - 1)
                    zp = pnrm.tile([128, QT + SCC], bf16, tag="zp",
                                   name=f"zp{t}{qt}")
                    tmp1 = pnrm.tile([64, QT], bf16, tag="tmp1", name=f"tm{t}{qt}")
                    for i in range(2):
                        dro = pnrm.tile([1, QT], f32, tag=f"dr{i}", name=f"dr{t}{qt}{i}")
                        nc.vector.tensor_copy(dro, av[i][DH:DH + 1, :])
                        nc.sync.dma_start(out=rs_d[t, i, qt:qt + 1, :], in_=dro)
                        rb = pnrm.tile([64, QT], f32, tag=f"rb{i}", name=f"rb{t}{qt}{i}")
                        nc.gpsimd.dma_start(
                            out=rb,
                            in_=rs_d[t, i, qt:qt + 1, :].to_broadcast([64, QT]))
                        nc.vector.reciprocal_approx_fast(rb, rb)
                        if i == 0:
                            nc.vector.tensor_mul(zp[0:64, 0:QT], av[0][0:DH, :], rb)
                            nc.vector.bn_stats(out=bnst[h0][:, qt, :],
                                               in_=zp[0:64, 0:QT])
                        else:
                            nc.vector.tensor_mul(tmp1, av[1][0:DH, :], rb)
                            nc.vector.bn_stats(out=bnst[h1][:, qt, :],
                                               in_=tmp1)
                            nc.gpsimd.dma_start(out=zp[64:128, 0:QT], in_=tmp1)

                    if qt == NQT - 1:
                        # pair-t GN stat fold (needs all 4 qt blocks)
                        for i, h in enumerate((h0, h1)):
                            mvh = pd.tile([64, 2], f32, tag="mv", bufs=2,
                                          name=f"mv{h}")
                            nc.vector.bn_aggr(out=mvh, in_=bnst[h])
                            stk = stk_all[h]
                            nc.vector.tensor_add(stk[:, 0:1], mvh[:, 0:1],
                                                 bv_sb[:, h:h + 1])
                            nc.vector.tensor_copy(stk[:, 1:2], mvh[:, 1:2])
                            nc.vector.tensor_mul(stk[:, 2:3], stk[:, 0:1],
                                                 stk[:, 0:1])

                    if last:
                        # ---- GN scalar tail: rstd via exp(-0.5*ln(var+eps)),
                        # same ACT table set as the softmax exps ----
                        scg = pd.tile([1, HPC, 3], f32, tag="scg", name="scg")
                        for h in range(HPC):
                            stp = pav.tile([1, 3], f32, tag="av0", name=f"stp{h}")
                            nc.tensor.matmul(stp, ones_sb, stk_all[h],
                                             start=True, stop=True)
                            nc.vector.tensor_copy(scg[:, h, :], stp)
                        e3 = pd.tile([1, HPC, 3], f32, tag="e3", name="e3")
                        nc.vector.tensor_scalar(out=e3, in0=scg, scalar1=1.0 / 64.0,
                                                scalar2=None, op0=ALU.mult)
                        m2 = pd.tile([1, HPC], f32, tag="m2", name="m2")
                        nc.vector.tensor_mul(m2, e3[:, :, 0], e3[:, :, 0])
                        nc.vector.tensor_add(vr_all, e3[:, :, 1], e3[:, :, 2])
                        nc.vector.tensor_tensor(out=vr_all, in0=vr_all, in1=m2,
                                                op=ALU.subtract)
                        eps_t = pd.tile([1, 1], f32, tag="eps", name="eps_t")
                        nc.vector.memset(eps_t, EPS)
                        lnv = pd.tile([1, HPC], f32, tag="lnv", name="lnv")
                        nc.scalar.activation(lnv, vr_all, AF.Ln, bias=eps_t)
                        rr = pd.tile([1, HPC], f32, tag="rr", name="rr")
                        nc.scalar.activation(rr, lnv, AF.Exp, scale=-0.5)
                        # parity-major payload order: [M0,M2,M1,M3, r0,r2,r1,r3]
                        for j, h in enumerate((0, 2, 1, 3)):
                            nc.vector.tensor_copy(msc_all[:, j:j + 1], e3[:, h, 0:1])
                            nc.vector.tensor_copy(msc_all[:, HPC + j:HPC + j + 1],
                                                  rr[:, h:h + 1])
                        nc.vector.tensor_copy(out=zp[0:1, QT:QT + SCC],
                                              in_=msc_all[0:1, :].bitcast(bf16))
                        nc.sync.dma_start(out=ag_in_last[:, :], in_=zp)
                        gather(ag_in_last[:], ag_out_last[:])
                        for g in range(4):
                            nc.sync.dma_start(
                                out=nrmg_sb[:, 2 * g + 1, 3 * QT:S],
                                in_=ag_out_last[g, :, 0:QT])
                    else:
                        nc.sync.dma_start(out=ag_in[t, qt], in_=zp[:, 0:QT])
                        gather(ag_in[t, qt], ag_out[t, qt])
                        for g in range(4):
                            nc.sync.dma_start(
                                out=nrmg_sb[:, 2 * g + t, qt * QT:(qt + 1) * QT],
                                in_=ag_out[t, qt, g])

        # ---- Phase E: fold GN affine into Wo, column-parallel out-proj ----
        with tc.tile_pool(name="pg", bufs=1) as pg, \
             tc.tile_pool(name="pf", bufs=4, space="PSUM") as pf, \
             tc.tile_pool(name="pystage", bufs=2) as pystage:
            # gathered scalars: [4 groups, 8 f32] as bitcast bf16 rows
            sc16 = pg.tile([1, 4, SCC], bf16, tag="sc16")
            nc.sync.dma_start(
                out=sc16,
                in_=ag_out_last[:, 0:1, QT:QT + SCC].rearrange("g p c -> p g c"))
            # [1, 4, 8] f32 per group: (M0,M2,M1,M3, r0,r2,r1,r3)
            scf = sc16[:, :, :].bitcast(f32)
            # per out-partition half o: values for chunks c=(g,t) are the
            # parity-o pair within each group -- contiguous slices
            rstg = pg.tile([1, 2, 4, 2], f32, tag="rstg")
            mstg = pg.tile([1, 2, 4, 2], f32, tag="mstg")
            for o in range(2):
                nc.vector.tensor_copy(out=rstg[:, o, :, :],
                                      in_=scf[:, :, HPC + 2 * o:HPC + 2 * o + 2])
                nc.vector.tensor_copy(out=mstg[:, o, :, :],
                                      in_=scf[:, :, 2 * o:2 * o + 2])
            s2p = pf.tile([128, NDC], f32, tag="s2p", bufs=1, name="s2p")
            mcp = pf.tile([128, NDC], f32, tag="mcp", bufs=1, name="mcp")
            for o in range(2):
                nc.tensor.matmul(s2p[64 * o:64 * (o + 1), :], ones2_sb[0:1, :],
                                 rstg[:, o, :, :], start=True, stop=True)
                nc.tensor.matmul(mcp[64 * o:64 * (o + 1), :], ones2_sb[0:1, :],
                                 mstg[:, o, :, :], start=True, stop=True)
            s2c = pg.tile([128, NDC], f32, tag="s2c")
            nc.vector.tensor_copy(s2c, s2p)
            bvg = pg.tile([128, NDC], f32, tag="bvg")
            nc.sync.dma_start(out=bvg, in_=bvf_d[:].rearrange("(c p) -> p c", p=128))
            mcs = pg.tile([128, NDC], f32, tag="mcs")
            nc.vector.tensor_tensor(out=mcs, in0=mcp, in1=bvg, op=ALU.subtract)
            mvec = pg.tile([128, NDC], bf16, tag="mvec")
            nc.vector.tensor_mul(mvec, mcs, s2c)

            # wo_scaled[p, (c,n)] = r_head(p,c) * wo ; cst[n] = sum_p M*r*wo
            wos = pg.tile([128, NDC, CW], bf16, tag="wos")
            for c in range(NDC):
                nc.vector.tensor_scalar(out=wos[:, c, :], in0=wo_sb[:, c, :],
                                        scalar1=s2c[:, c:c + 1], scalar2=None,
                                        op0=ALU.mult)
            cstp = pf.tile([1, CW], f32, tag="cst", bufs=1, name="cstp")
            for c in range(NDC):
                nc.tensor.matmul(cstp, mvec[:, c:c + 1], wo_sb[:, c, :],
                                 start=(c == 0), stop=(c == NDC - 1))
            brow = pg.tile([1, CW], bf16, tag="brow")
            nc.vector.tensor_tensor(out=brow, in0=bor_sb, in1=cstp, op=ALU.subtract)

            for nt in range(2):
                ystage = pystage.tile([128, S], f32, tag="ys", name=f"ys{nt}")
                yps = [pf.tile([128, QT], f32, tag="y", name=f"yp{nt}{st}")
                       for st in range(NQT)]
                for c in range(NDC):
                    for st in range(NQT):
                        nc.tensor.matmul(yps[st], wos[:, c, nt * 128:(nt + 1) * 128],
                                         nrmg_sb[:, c, st * QT:(st + 1) * QT],
                                         start=(c == 0), stop=False)
                for st in range(NQT):
                    nc.tensor.matmul(yps[st], brow[:, nt * 128:(nt + 1) * 128],
                                     onesrow_sb, start=False, stop=True)
                    nc.scalar.activation(ystage[:, st * QT:(st + 1) * QT], yps[st],
                                         AF.Copy)
                nc.sync.dma_start(out=y_d[nt, :, :], in_=ystage)

    nc.compile()
    return nc


def _get_nc():
    if "nc" not in _cache:
        _cache["nc"] = _build()
    return _cache["nc"]


def _host_prep(x, Wq, bq, Wk, bk, Wv, bv, Wo, bo, lq1, lk1, lq2, lk2, gn_w, gn_b):
    x = np.asarray(x, np.float32)
    lam = (np.exp((np.asarray(lq1) * np.asarray(lk1)).sum(-1))
           - np.exp((np.asarray(lq2) * np.asarray(lk2)).sum(-1)) + LAMBDA_INIT)
    qscale = (DH ** -0.5) * lam
    Wq_eff = (np.asarray(Wq).reshape(D, H, DH) * qscale[None, :, None]).reshape(D, D)
    bq_eff = (np.asarray(bq).reshape(H, DH) * qscale[:, None]).reshape(D)
    gw = np.asarray(gn_w).reshape(D)
    gb = np.asarray(gn_b).reshape(D)
    Wo_eff = np.asarray(Wo) * gw[:, None]
    bo_eff = np.asarray(bo) + gb @ np.asarray(Wo)

    # Gathered-row order (chunk (g,t), partition (o,dh) -> head 4g+2t+o) is
    # exactly the original row-major head order, so Wo_eff rows need no
    # permutation.  (bk is dropped: q.bk is constant along the softmax axis.)
    xT = np.ascontiguousarray(x.transpose(0, 2, 1))  # [B, D, S]
    bf = ml_dtypes.bfloat16

    in_maps = []
    for c in range(N_CORES):
        b, hg = c // 4, c % 4
        cs = slice(CW * hg, CW * (hg + 1))
        in_maps.append({
            "xt": np.ascontiguousarray(xT[b]).astype(bf),
            "wq": np.ascontiguousarray(Wq_eff[:, cs]).astype(bf),
            "wk": np.ascontiguousarray(np.asarray(Wk)[:, cs]).astype(bf),
            "wv": np.ascontiguousarray(np.asarray(Wv)[:, cs]).astype(bf),
            "wo": np.ascontiguousarray(Wo_eff[:, cs]).astype(bf),
            "bq": np.ascontiguousarray(bq_eff[cs]).astype(bf),
            "bv": np.ascontiguousarray(np.asarray(bv)[cs]).astype(np.float32),
            "bvf": np.ascontiguousarray(np.asarray(bv)).astype(np.float32),
            "bo": np.ascontiguousarray(bo_eff[cs]).astype(bf),
        })
    return in_maps


def _host_gather(outs):
    # core c=4b+hg produced output columns [256*hg, 256*(hg+1)) as [2,128,S]
    yT = np.empty((B, D, S), np.float32)
    for b in range(B):
        for hg in range(4):
            q = np.asarray(outs[4 * b + hg]["y"]).reshape(CW, S)
            yT[b, CW * hg:CW * (hg + 1), :] = q
    return np.ascontiguousarray(yT.transpose(0, 2, 1))


def kernel(x, Wq, bq, Wk, bk, Wv, bv, Wo, bo, lq1, lk1, lq2, lk2, gn_w, gn_b):
    from concourse.bass_utils import run_bass_kernel_spmd

    in_maps = _host_prep(x, Wq, bq, Wk, bk, Wv, bv, Wo, bo,
                         lq1, lk1, lq2, lk2, gn_w, gn_b)
    nc = _get_nc()
    res = run_bass_kernel_spmd(nc, in_maps, core_ids=list(range(N_CORES)))
    return _host_gather(res.results)


# revision 14
# speedup vs baseline: 1.1838x; 1.1386x over previous
"""Multi-head differential attention on 8 Trainium2 NeuronCores.

Sharding: core c -> batch c//4, head-group c%4 (4 of 16 heads).

v2 pipeline (vs baseline): the softmax exp stream on the Scalar engine is
the per-core floor (~128us), so everything is scheduled around keeping it
fed from t~14us onward:
  - prefix computes only q/k of head-pair 0; V and pair-1 q/k are emitted
    as PE filler inside pair-0's attention loop (the PE has slack while
    ACT chews exps).
  - attention inner loop is kt-pipelined: score pair (row-tiled 64x128
    auto-tiles) -> exp of [128,1024] PSUM tile -> e_sb (bf16, 16 kt tiles
    per (t,qt)) -> deferred AV (lag 2) accumulating z+denominator via the
    DH+1 ones-row.
  - softmax-normalize + GroupNorm stats + gather payload happen per
    (t,qt), and the z AllGather is split into 8 per-(t,qt) chunks that
    pipeline on the CC rings under the attention phase (the monolithic
    per-pair gathers were 2x57us, mostly exposed).
  - k-bias is dropped entirely (constant along the softmax axis), lambda
    and softmax scale are folded into Wq/bq, GroupNorm affine into Wo/bo
    on host; rstd uses exp(-0.5*ln(var+eps)) so the whole kernel needs
    one ACT table set (no mid-kernel sqrt table switch).
Each core then runs a column-parallel out-projection producing a
256-column slice of the output, assembled on host.
"""

import numpy as np
import ml_dtypes

B, S, D, H, DH = 2, 2048, 1024, 16, 64
HPC = 4            # heads per core
CW = HPC * DH      # attention columns per core (256)
EPS = 1e-5
LAMBDA_INIT = 0.8
N_CORES = 8
SCC = 16           # scalar payload columns (8 f32 as 16 bf16)
QT = 512           # q-block per (t, qt)
NQT = 4
NKT = 16
NDC = 8

_cache = {}


def _build(with_collective=True):
    from contextlib import ExitStack
    import concourse.bass as bass
    from concourse import bacc
    import concourse.tile as tile
    import concourse.mybir as mybir

    f32 = mybir.dt.float32
    bf16 = mybir.dt.bfloat16
    AF = mybir.ActivationFunctionType
    ALU = mybir.AluOpType

    nc = bacc.Bacc("TRN2", target_bir_lowering=False, debug=False,
                   num_devices=N_CORES)

    xt_d = nc.dram_tensor("xt", [D, S], bf16, kind="ExternalInput")
    wq_d = nc.dram_tensor("wq", [D, CW], bf16, kind="ExternalInput")
    wk_d = nc.dram_tensor("wk", [D, CW], bf16, kind="ExternalInput")
    wv_d = nc.dram_tensor("wv", [D, CW], bf16, kind="ExternalInput")
    wo_d = nc.dram_tensor("wo", [D, CW], bf16, kind="ExternalInput")
    bq_d = nc.dram_tensor("bq", [CW], bf16, kind="ExternalInput")
    bv_d = nc.dram_tensor("bv", [CW], f32, kind="ExternalInput")
    bvf_d = nc.dram_tensor("bvf", [D], f32, kind="ExternalInput")
    bo_d = nc.dram_tensor("bo", [CW], bf16, kind="ExternalInput")
    y_d = nc.dram_tensor("y", [2, 128, S], f32, kind="ExternalOutput")

    rs_d = nc.dram_tensor("rs_scratch", [2, 2, NQT, QT], f32)
    warm_in = nc.dram_tensor("warm_in", [1, 64], bf16)
    warm_out = nc.dram_tensor("warm_out", [4, 1, 64], bf16)
    # 4 gather chunks: one per (t, qt-pair) of 1024 cols; last carries scalars
    ag_in = nc.dram_tensor("ag_in", [3, 128, 2 * QT], bf16)
    ag_in_last = nc.dram_tensor("ag_in_last", [128, 2 * QT + SCC], bf16)
    ag_out = nc.dram_tensor("ag_out", [3, 4, 128, 2 * QT], bf16)
    ag_out_last = nc.dram_tensor("ag_out_last", [4, 128, 2 * QT + SCC], bf16)

    GROUPS = [[0, 1, 2, 3], [4, 5, 6, 7]]

    def gather(in_ap, out_ap):
        if with_collective:
            nc.gpsimd.collective_compute(
                "AllGather", ALU.bypass, replica_groups=GROUPS,
                ins=[in_ap.opt()], outs=[out_ap.opt()])
        else:
            for g in range(4):
                nc.sync.dma_start(out=out_ap[g], in_=in_ap)

    with ExitStack() as ctx:
        tc = ctx.enter_context(tile.TileContext(nc))
        const = ctx.enter_context(tc.tile_pool(name="const", bufs=1))
        big = ctx.enter_context(tc.tile_pool(name="big", bufs=1))

        pxt = ctx.enter_context(tc.tile_pool(name="pxt", bufs=1))
        # warm-up collective input first: its gather absorbs the cross-core
        # rendezvous barrier, so it must not queue behind the bulk input DMAs
        wu_sb = pxt.tile([1, 64], bf16, tag="wu", name="wu")
        nc.vector.memset(wu_sb, 0.0)
        nc.sync.dma_start(out=warm_in[:, :], in_=wu_sb)

        xt_sb = [pxt.tile([128, S], bf16, tag=f"xt{c}", name=f"xt{c}")
                 for c in range(NDC)]
        # split across two DMA queues for bandwidth
        for c in range(NDC):
            eng = nc.sync if c % 2 == 0 else nc.gpsimd
            eng.dma_start(out=xt_sb[c], in_=xt_d[c * 128:(c + 1) * 128, :])

        # early dummy collective: absorbs the cross-core rendezvous barrier
        # under the prefix instead of delaying the first real gather
        gather(warm_in[:], warm_out[:])

        # ---- constants ----
        wq_sb = const.tile([128, NDC, CW], bf16, tag="wq")
        wk_sb = const.tile([128, NDC, CW], bf16, tag="wk")
        wv_sb = const.tile([128, NDC, CW], bf16, tag="wv")
        wo_sb = const.tile([128, NDC, CW], bf16, tag="wo")
        nc.sync.dma_start(out=wq_sb, in_=wq_d[:, :].rearrange("(c p) n -> p c n", p=128))
        nc.sync.dma_start(out=wk_sb, in_=wk_d[:, :].rearrange("(c p) n -> p c n", p=128))
        nc.sync.dma_start(out=wv_sb, in_=wv_d[:, :].rearrange("(c p) n -> p c n", p=128))
        nc.sync.dma_start(out=wo_sb, in_=wo_d[:, :].rearrange("(c p) n -> p c n", p=128))

        bqr_sb = const.tile([1, CW], bf16, tag="bqr")
        bor_sb = const.tile([1, CW], bf16, tag="bor")
        nc.sync.dma_start(out=bqr_sb, in_=bq_d[:].rearrange("(a n) -> a n", a=1))
        nc.sync.dma_start(out=bor_sb, in_=bo_d[:].rearrange("(a n) -> a n", a=1))
        bv0_sb = const.tile([64, HPC], f32, tag="bv0")
        nc.sync.dma_start(out=bv0_sb, in_=bv_d[:].rearrange("(h p) -> p h", p=64))
        bv_sb = const.tile([64, HPC], f32, tag="bv")
        nc.vector.tensor_copy(bv_sb, bv0_sb)  # pre-touch: keep deps DVE-local

        onesrow_sb = const.tile([1, QT], bf16, tag="onesrow")
        nc.vector.memset(onesrow_sb, 1.0)
        ones_sb = const.tile([64, 1], f32, tag="ones")
        nc.vector.memset(ones_sb, 1.0)
        ones2_sb = const.tile([2, 64], f32, tag="ones2")
        nc.vector.memset(ones2_sb, 1.0)

        qT_sb = big.tile([128, 2, S], bf16, tag="qT")   # pair t: head 2t rows 0:64
        kT_sb = big.tile([128, 2, S], bf16, tag="kT")
        v_sb = [big.tile([128, NKT, DH + 1], bf16, tag=f"v{h}", name=f"v{h}")
                for h in range(HPC)]
        for h in range(HPC):
            nc.vector.memset(v_sb[h][:, :, DH:DH + 1], 1.0)
        nrmg_sb = big.tile([128, NDC, S], bf16, tag="nrmg")

        with tc.tile_pool(name="pqk", bufs=2, space="PSUM") as pqk, \
             tc.tile_pool(name="psc", bufs=2, space="PSUM") as psc, \
             tc.tile_pool(name="pav", bufs=1, space="PSUM") as pav, \
             tc.tile_pool(name="pe", bufs=1) as pe_pool, \
             tc.tile_pool(name="pnrm", bufs=2) as pnrm, \
             tc.tile_pool(name="pd", bufs=1) as pd:

            bnst = [pd.tile([64, NQT, 6], f32, tag=f"bn{h}", name=f"bnst{h}")
                    for h in range(HPC)]
            vr_all = pd.tile([1, HPC], f32, tag="vr_all", name="vr_all")
            msc_all = pd.tile([1, 2 * HPC], f32, tag="msc", name="msc_all")
            stk_all = [pd.tile([64, 3], f32, tag=f"stk{h}", name=f"stk{h}")
                       for h in range(HPC)]

            def qk_proj_st(t, st, w_sb, bias):
                ps = pqk.tile([128, QT], f32, tag="qk", name=f"qk{t}{st}{w_sb.tensor.name}")
                for c in range(NDC):
                    nc.tensor.matmul(ps, w_sb[:, c, t * 128:(t + 1) * 128],
                                     xt_sb[c][:, st * QT:(st + 1) * QT],
                                     start=(c == 0),
                                     stop=(not bias and c == NDC - 1))
                if bias:
                    nc.tensor.matmul(ps, bqr_sb[:, t * 128:(t + 1) * 128],
                                     onesrow_sb, start=False, stop=True)
                dst = qT_sb if bias else kT_sb
                nc.vector.tensor_copy(out=dst[:, t, st * QT:(st + 1) * QT], in_=ps)

            def v_unit(pair, st):
                # V for heads (2*pair, 2*pair+1) at key block st
                ps = pqk.tile([128, QT], f32, tag="qk", name=f"v{pair}{st}")
                for c in range(NDC):
                    nc.tensor.matmul(ps[:, 0:128],
                                     xt_sb[c][:, st * 128:(st + 1) * 128],
                                     wv_sb[:, c, pair * 128:(pair + 1) * 128],
                                     start=(c == 0), stop=(c == NDC - 1))
                for i in range(2):
                    h = 2 * pair + i
                    nc.vector.tensor_copy(out=v_sb[h][:, st, 0:DH],
                                          in_=ps[:, i * DH:(i + 1) * DH])

            # ---- prefix: q/k for pair 0 only ----
            for st in range(NQT):
                qk_proj_st(0, st, wq_sb, True)
            for st in range(NQT):
                qk_proj_st(0, st, wk_sb, False)

            # deferred PE work, emitted as filler inside pair-0's loop
            fillers = []
            for st in range(NKT):
                fillers.append((1100, lambda st=st: v_unit(0, st)))
            for st in range(NQT):
                fillers.append((4200, lambda st=st: qk_proj_st(1, st, wk_sb, False)))
            for st in range(NQT):
                fillers.append((4700, lambda st=st: qk_proj_st(1, st, wq_sb, True)))
            for st in range(NKT):
                fillers.append((1100, lambda st=st: v_unit(1, st)))
            filler_total = sum(c for c, _ in fillers)
            filler_spent = 0
            filler_budget = 0.0
            zp_chunks = {}

            for t in range(2):
                h0, h1 = 2 * t, 2 * t + 1
                for qt in range(NQT):
                    e_sb = pe_pool.tile([128, NKT * 2 * QT], bf16,
                                        tag=f"e{qt % 2}", name=f"e{t}{qt}")
                    av = [pav.tile([DH + 1, QT], f32, tag=f"av{i}",
                                   name=f"av{t}{qt}{i}") for i in range(2)]

                    def emit_av(kt):
                        for i in range(2):
                            nc.tensor.matmul(
                                av[i], v_sb[2 * t + i][:, kt, :],
                                e_sb[:, kt * 1024 + i * QT:kt * 1024 + (i + 1) * QT],
                                start=(kt == 0), stop=(kt == NKT - 1))

                    for kt in range(NKT):
                        sps = psc.tile([128, 2 * QT], f32, tag="s",
                                       name=f"s{t}{qt}{kt}")
                        for o in range(2):
                            nc.tensor.matmul(
                                sps[:, o * QT:(o + 1) * QT],
                                kT_sb[64 * o:64 * (o + 1), t, kt * 128:(kt + 1) * 128],
                                qT_sb[64 * o:64 * (o + 1), t, qt * QT:(qt + 1) * QT],
                                start=True, stop=True)
                        nc.scalar.activation(
                            e_sb[:, kt * 1024:(kt + 1) * 1024], sps, AF.Exp)
                        if t == 0:
                            filler_budget += filler_total / 64.0
                            while fillers and filler_spent < filler_budget:
                                cost, fn = fillers.pop(0)
                                fn()
                                filler_spent += cost
                        if kt >= 2:
                            emit_av(kt - 2)
                    emit_av(NKT - 2)
                    emit_av(NKT - 1)

                    # ---- per-(t,qt) softmax normalize + GN stats ----
                    last = (t == 1 and qt == NQT - 1)
                    # evacuate av PSUM to SBUF immediately: the normalize
                    # chain below has DMA round-trip latency, and next qt's
                    # AVs head-of-line block the PE until av is released
                    zc = [pnrm.tile([DH + 1, QT], f32, tag=f"zc{i}",
                                    name=f"zc{t}{qt}{i}") for i in range(2)]
                    nc.vector.tensor_copy(zc[0], av[0])
                    nc.vector.tensor_copy(zc[1], av[1])

                    ci = 2 * t + qt // 2
                    cb = (qt % 2) * QT  # column base within gather chunk
                    if qt % 2 == 0:
                        zp = pnrm.tile([128, 2 * QT + SCC], bf16, tag="zp",
                                       name=f"zp{ci}")
                        zp_chunks[ci] = zp
                    else:
                        zp = zp_chunks[ci]
                    tmp1 = pnrm.tile([64, QT], bf16, tag="tmp1", name=f"tm{t}{qt}")
                    for i in range(2):
                        nc.sync.dma_start(out=rs_d[t, i, qt:qt + 1, :],
                                          in_=zc[i][DH:DH + 1, :])
                        rb = pnrm.tile([64, QT], f32, tag=f"rb{i}", name=f"rb{t}{qt}{i}")
                        nc.gpsimd.dma_start(
                            out=rb,
                            in_=rs_d[t, i, qt:qt + 1, :].to_broadcast([64, QT]))
                        nc.vector.reciprocal_approx_fast(rb, rb)
                        if i == 0:
                            nc.vector.tensor_mul(zp[0:64, cb:cb + QT],
                                                 zc[0][0:DH, :], rb)
                            nc.vector.bn_stats(out=bnst[h0][:, qt, :],
                                               in_=zp[0:64, cb:cb + QT])
                        else:
                            nc.vector.tensor_mul(tmp1, zc[1][0:DH, :], rb)
                            nc.vector.bn_stats(out=bnst[h1][:, qt, :],
                                               in_=tmp1)
                            nc.gpsimd.dma_start(out=zp[64:128, cb:cb + QT],
                                                in_=tmp1)

                    if qt == NQT - 1:
                        # pair-t GN stat fold (needs all 4 qt blocks)
                        for i, h in enumerate((h0, h1)):
                            mvh = pd.tile([64, 2], f32, tag="mv", bufs=2,
                                          name=f"mv{h}")
                            nc.vector.bn_aggr(out=mvh, in_=bnst[h])
                            stk = stk_all[h]
                            nc.vector.tensor_add(stk[:, 0:1], mvh[:, 0:1],
                                                 bv_sb[:, h:h + 1])
                            nc.vector.tensor_copy(stk[:, 1:2], mvh[:, 1:2])
                            nc.vector.tensor_mul(stk[:, 2:3], stk[:, 0:1],
                                                 stk[:, 0:1])

                    if last:
                        # ---- GN scalar tail: rstd via exp(-0.5*ln(var+eps)),
                        # same ACT table set as the softmax exps ----
                        scg = pd.tile([1, HPC, 3], f32, tag="scg", name="scg")
                        for h in range(HPC):
                            stp = pav.tile([1, 3], f32, tag="av0", name=f"stp{h}")
                            nc.tensor.matmul(stp, ones_sb, stk_all[h],
                                             start=True, stop=True)
                            nc.vector.tensor_copy(scg[:, h, :], stp)
                        e3 = pd.tile([1, HPC, 3], f32, tag="e3", name="e3")
                        nc.vector.tensor_scalar(out=e3, in0=scg, scalar1=1.0 / 64.0,
                                                scalar2=None, op0=ALU.mult)
                        m2 = pd.tile([1, HPC], f32, tag="m2", name="m2")
                        nc.vector.tensor_mul(m2, e3[:, :, 0], e3[:, :, 0])
                        nc.vector.tensor_add(vr_all, e3[:, :, 1], e3[:, :, 2])
                        nc.vector.tensor_tensor(out=vr_all, in0=vr_all, in1=m2,
                                                op=ALU.subtract)
                        eps_t = pd.tile([1, 1], f32, tag="eps", name="eps_t")
                        nc.vector.memset(eps_t, EPS)
                        lnv = pd.tile([1, HPC], f32, tag="lnv", name="lnv")
                        nc.scalar.activation(lnv, vr_all, AF.Ln, bias=eps_t)
                        rr = pd.tile([1, HPC], f32, tag="rr", name="rr")
                        nc.scalar.activation(rr, lnv, AF.Exp, scale=-0.5)
                        # parity-major payload order: [M0,M2,M1,M3, r0,r2,r1,r3]
                        for j, h in enumerate((0, 2, 1, 3)):
                            nc.vector.tensor_copy(msc_all[:, j:j + 1], e3[:, h, 0:1])
                            nc.vector.tensor_copy(msc_all[:, HPC + j:HPC + j + 1],
                                                  rr[:, h:h + 1])
                        nc.vector.tensor_copy(out=zp[0:1, 2 * QT:2 * QT + SCC],
                                              in_=msc_all[0:1, :].bitcast(bf16))
                        nc.sync.dma_start(out=ag_in_last[:, :], in_=zp)
                        gather(ag_in_last[:], ag_out_last[:])
                    elif qt % 2 == 1:
                        nc.sync.dma_start(out=ag_in[ci], in_=zp[:, 0:2 * QT])
                        gather(ag_in[ci], ag_out[ci])

            # nrmg loads after all chunks are in flight: chunks 0-2 are
            # long done (no queue-blocking wait); only the last chunk's
            # loads wait, inside its unavoidable gather window
            for ci in range(3):
                t2, qp = divmod(ci, 2)
                for g in range(4):
                    eng = nc.sync if g % 2 == 0 else nc.gpsimd
                    eng.dma_start(
                        out=nrmg_sb[:, 2 * g + t2, qp * 2 * QT:(qp + 1) * 2 * QT],
                        in_=ag_out[ci, g])
            for g in range(4):
                eng = nc.sync if g % 2 == 0 else nc.gpsimd
                eng.dma_start(out=nrmg_sb[:, 2 * g + 1, 2 * QT:4 * QT],
                              in_=ag_out_last[g, :, 0:2 * QT])

        # ---- Phase E: fold GN affine into Wo, column-parallel out-proj ----
        with tc.tile_pool(name="pg", bufs=1) as pg, \
             tc.tile_pool(name="pf", bufs=4, space="PSUM") as pf, \
             tc.tile_pool(name="pystage", bufs=2) as pystage:
            # gathered scalars: [4 groups, 8 f32] as bitcast bf16 rows
            sc16 = pg.tile([1, 4, SCC], bf16, tag="sc16")
            nc.sync.dma_start(
                out=sc16,
                in_=ag_out_last[:, 0:1, 2 * QT:2 * QT + SCC].rearrange("g p c -> p g c"))
            # [1, 4, 8] f32 per group: (M0,M2,M1,M3, r0,r2,r1,r3)
            scf = sc16[:, :, :].bitcast(f32)
            # per out-partition half o: values for chunks c=(g,t) are the
            # parity-o pair within each group -- contiguous slices
            rstg = pg.tile([1, 2, 4, 2], f32, tag="rstg")
            mstg = pg.tile([1, 2, 4, 2], f32, tag="mstg")
            for o in range(2):
                nc.vector.tensor_copy(out=rstg[:, o, :, :],
                                      in_=scf[:, :, HPC + 2 * o:HPC + 2 * o + 2])
                nc.vector.tensor_copy(out=mstg[:, o, :, :],
                                      in_=scf[:, :, 2 * o:2 * o + 2])
            s2p = pf.tile([128, NDC], f32, tag="s2p", bufs=1, name="s2p")
            mcp = pf.tile([128, NDC], f32, tag="mcp", bufs=1, name="mcp")
            for o in range(2):
                nc.tensor.matmul(s2p[64 * o:64 * (o + 1), :], ones2_sb[0:1, :],
                                 rstg[:, o, :, :], start=True, stop=True)
                nc.tensor.matmul(mcp[64 * o:64 * (o + 1), :], ones2_sb[0:1, :],
                                 mstg[:, o, :, :], start=True, stop=True)
            s2c = pg.tile([128, NDC], f32, tag="s2c")
            nc.vector.tensor_copy(s2c, s2p)
            bvg = pg.tile([128, NDC], f32, tag="bvg")
            nc.sync.dma_start(out=bvg, in_=bvf_d[:].rearrange("(c p) -> p c", p=128))
            mcs = pg.tile([128, NDC], f32, tag="mcs")
            nc.vector.tensor_tensor(out=mcs, in0=mcp, in1=bvg, op=ALU.subtract)
            mvec = pg.tile([128, NDC], bf16, tag="mvec")
            nc.vector.tensor_mul(mvec, mcs, s2c)

            # wo_scaled[p, (c,n)] = r_head(p,c) * wo ; cst[n] = sum_p M*r*wo
            wos = pg.tile([128, NDC, CW], bf16, tag="wos")
            for c in range(NDC):
                nc.vector.tensor_scalar(out=wos[:, c, :], in0=wo_sb[:, c, :],
                                        scalar1=s2c[:, c:c + 1], scalar2=None,
                                        op0=ALU.mult)
            cstp = pf.tile([1, CW], f32, tag="cst", bufs=1, name="cstp")
            for c in range(NDC):
                nc.tensor.matmul(cstp, mvec[:, c:c + 1], wo_sb[:, c, :],
                                 start=(c == 0), stop=(c == NDC - 1))
            brow = pg.tile([1, CW], bf16, tag="brow")
            nc.vector.tensor_tensor(out=brow, in0=bor_sb, in1=cstp, op=ALU.subtract)

            for nt in range(2):
                ystage = pystage.tile([128, S], f32, tag="ys", name=f"ys{nt}")
                yps = [pf.tile([128, QT], f32, tag="y", name=f"yp{nt}{st}")
                       for st in range(NQT)]
                for c in range(NDC):
                    for st in range(NQT):
                        nc.tensor.matmul(yps[st], wos[:, c, nt * 128:(nt + 1) * 128],
                                         nrmg_sb[:, c, st * QT:(st + 1) * QT],
                                         start=(c == 0), stop=False)
                for st in range(NQT):
                    nc.tensor.matmul(yps[st], brow[:, nt * 128:(nt + 1) * 128],
                                     onesrow_sb, start=False, stop=True)
                    nc.scalar.activation(ystage[:, st * QT:(st + 1) * QT], yps[st],
                                         AF.Copy)
                nc.sync.dma_start(out=y_d[nt, :, :], in_=ystage)

    nc.compile()
    return nc


def _get_nc():
    if "nc" not in _cache:
        _cache["nc"] = _build()
    return _cache["nc"]


def _host_prep(x, Wq, bq, Wk, bk, Wv, bv, Wo, bo, lq1, lk1, lq2, lk2, gn_w, gn_b):
    x = np.asarray(x, np.float32)
    lam = (np.exp((np.asarray(lq1) * np.asarray(lk1)).sum(-1))
           - np.exp((np.asarray(lq2) * np.asarray(lk2)).sum(-1)) + LAMBDA_INIT)
    qscale = (DH ** -0.5) * lam
    Wq_eff = (np.asarray(Wq).reshape(D, H, DH) * qscale[None, :, None]).reshape(D, D)
    bq_eff = (np.asarray(bq).reshape(H, DH) * qscale[:, None]).reshape(D)
    gw = np.asarray(gn_w).reshape(D)
    gb = np.asarray(gn_b).reshape(D)
    Wo_eff = np.asarray(Wo) * gw[:, None]
    bo_eff = np.asarray(bo) + gb @ np.asarray(Wo)

    # Gathered-row order (chunk (g,t), partition (o,dh) -> head 4g+2t+o) is
    # exactly the original row-major head order, so Wo_eff rows need no
    # permutation.  (bk is dropped: q.bk is constant along the softmax axis.)
    xT = np.ascontiguousarray(x.transpose(0, 2, 1))  # [B, D, S]
    bf = ml_dtypes.bfloat16

    in_maps = []
    for c in range(N_CORES):
        b, hg = c // 4, c % 4
        cs = slice(CW * hg, CW * (hg + 1))
        in_maps.append({
            "xt": np.ascontiguousarray(xT[b]).astype(bf),
            "wq": np.ascontiguousarray(Wq_eff[:, cs]).astype(bf),
            "wk": np.ascontiguousarray(np.asarray(Wk)[:, cs]).astype(bf),
            "wv": np.ascontiguousarray(np.asarray(Wv)[:, cs]).astype(bf),
            "wo": np.ascontiguousarray(Wo_eff[:, cs]).astype(bf),
            "bq": np.ascontiguousarray(bq_eff[cs]).astype(bf),
            "bv": np.ascontiguousarray(np.asarray(bv)[cs]).astype(np.float32),
            "bvf": np.ascontiguousarray(np.asarray(bv)).astype(np.float32),
            "bo": np.ascontiguousarray(bo_eff[cs]).astype(bf),
        })
    return in_maps


def _host_gather(outs):
    # core c=4b+hg produced output columns [256*hg, 256*(hg+1)) as [2,128,S]
    yT = np.empty((B, D, S), np.float32)
    for b in range(B):
        for hg in range(4):
            q = np.asarray(outs[4 * b + hg]["y"]).reshape(CW, S)
            yT[b, CW * hg:CW * (hg + 1), :] = q
    return np.ascontiguousarray(yT.transpose(0, 2, 1))


def kernel(x, Wq, bq, Wk, bk, Wv, bv, Wo, bo, lq1, lk1, lq2, lk2, gn_w, gn_b):
    from concourse.bass_utils import run_bass_kernel_spmd

    in_maps = _host_prep(x, Wq, bq, Wk, bk, Wv, bv, Wo, bo,
                         lq1, lk1, lq2, lk2, gn_w, gn_b)
    nc = _get_nc()
    res = run_bass_kernel_spmd(nc, in_maps, core_ids=list(range(N_CORES)))
    return _host_gather(res.results)
